# revision 41
# baseline (speedup 1.0000x reference)
"""KoLeo loss kernel for Trainium2, 8 NeuronCores (SPMD), fp8 DoubleRow.

reference math:
    x = thought_vectors.reshape(-1, D)          # [N, D], N=8192, D=1024
    xn = x / max(||x||, 1e-12)
    sim = min(xn @ xn.T, 1.0)
    dist = sqrt(2 - 2*sim + 1e-4), diag -> inf
    loss = -mean(log(min_row_dist + 1e-8))

Key reductions:
  * dist is monotone-decreasing in sim, so only the row-max of the Gram
    matrix (diag excluded) is needed.
  * log(d + 1e-8) ~= 0.5*log(d^2) to ~1e-6 abs, so the final pass is a
    single Ln activation on the clamped row-max.

Implementation (SPMD: all cores run the identical program; core-relative
addressing uses the runtime partition-id register in dynamic-offset
DMAs, which keeps the instruction stream uniform):
  * Rows sharded across 8 cores (1024 each). Each core normalizes its
    shard scaled by 16, quantizes to fp8e4 (e4m3), transposes to [D, n]
    layout, AllGathers the fp8 transposed shards in two column halves
    (A, B), then computes Gram blocks with DoubleRow fp8 matmuls
    (2 k-tiles per instruction, 2x PE throughput; 512-col moving
    operand so LDWEIGHTS hides under the previous matmul's stream).
    Gram values are scaled by 256.
  * 16 column chunks of 512: the core's own rows AND a replicated copy
    of core (me+1)'s rows (second input tensor - replication is part of
    the sharding strategy) are prepped locally, giving FOUR resident
    chunks (own diag-masked via a structural -1024 diagonal tile) whose
    compute bridges the runtime's communicator barrier (~40-75us of
    core launch skew) and the whole AllGather-A latency - on a typical
    run the PE never idles between the local and gathered phases. The
    12 gathered chunks read the 6 remaining peer blocks with a
    core-relative rotation (peer = (me+blk) & 7) via runtime-offset
    DMAs, so no column block is ever computed twice and the final
    per-row reduction is a plain max over the 16-entry chunk-max table.
  * Schedule is latency-shaped around the collectives:
      - all gathered-rhs loads are issued up front (rhsp bufs=14 keeps
        every tile resident, so each chunk's matmuls wait only on its
        own transfers; a wrapping pool coalesced the waits and stalled
        the first matmul ~10us);
      - rhs loads ride the gpsimd/scalar DMA queues in quarter-K tiles
        (128KB), so the first matmul starts a quarter-transfer after
        AG_A lands and the sync queue's engine (contended by collective
        traffic) is avoided;
      - AG_B is issued only after the A-half loads, gated on the first
        gathered chunk's tiles via a dummy rewrite of the B staging
        buffer: launched back-to-back with AG_A its link traffic slows
        the A-phase rhs reads to ~30GB/s and stalls the PE ~10us. The
        gate's copies run on gpsimd - on the DVE queue they'd sit ahead
        of the chunk drains and block PSUM bank recycling until AG_A
        lands.
  * PSUM drains are direct DVE f32 row-max reductions (683ns/bank vs
    1048ns bank refill); the old scalar-copy+bf16-reduce alternate had
    1.4us serial latency, gated the PE, and saved no DVE time (reduce
    has no 2x bf16 mode).
"""

import numpy as np

_P = 128
_SCALE = 16.0          # fp8 pre-scale; gram values are scaled by 256
_NEG = -1024.0         # structural diag mask, dominates the +-256 range
_EPS_NORM = 1e-12
_EPS_DIST = 1e-4


def _build_program(ncores, NB, D):
    import concourse.bacc as bacc
    import concourse.mybir as mybir
    from concourse.bass import ds as bass_ds
    from concourse.tile import TileContext
    from concourse.masks import make_identity

    f32 = mybir.dt.float32
    bf16 = mybir.dt.bfloat16
    fp8 = mybir.dt.float8e4
    P = _P
    M_TILES = NB // P          # 8
    K_TILES = D // P           # 8
    HB = NB // 2               # 512, also the column-chunk width
    MH = M_TILES // 2
    CHUNK = HB
    # 2 own + 2 replicated-neighbor resident chunks + 12 gathered chunks
    NCHUNK = 4 + 2 * (ncores - 2)
    DR = mybir.MatmulPerfMode.DoubleRow

    nc = bacc.Bacc(
        "TRN2", target_bir_lowering=False, debug=False, num_devices=ncores
    )
    xs = nc.dram_tensor("xs", [NB, D], f32, kind="ExternalInput")
    # core (c+1)%8's rows, replicated into this core's inputs: their two
    # Gram chunks compute from local data during the AllGather-A wait,
    # shrinking the post-AllGather stream by two chunks
    xs2 = nc.dram_tensor("xs2", [NB, D], f32, kind="ExternalInput")
    out = nc.dram_tensor("out", [P, M_TILES], f32, kind="ExternalOutput")

    with TileContext(nc) as tc:
        # one partition-id register load per engine, up front, so the
        # rotated-gather DMA offsets don't re-load it mid-stream
        nc.cache_partition_id()
        with (
            tc.tile_pool(name="consts", bufs=1) as consts,
            tc.tile_pool(name="dram", bufs=1, space="DRAM") as dram,
            tc.tile_pool(name="small", bufs=4) as small,
        ):
            identity = consts.tile([P, P], bf16)
            make_identity(nc, identity)
            diagneg = consts.tile([P, P], f32)
            nc.gpsimd.memset(diagneg, 0.0)
            nc.gpsimd.affine_select(
                out=diagneg,
                in_=diagneg,
                compare_op=mybir.AluOpType.not_equal,
                fill=_NEG,
                base=0,
                pattern=[[-1, P]],
                channel_multiplier=1,
            )
            # resident transposed fp8 shard halves: [P(d_lo), k(d_hi), n]
            xnT_A = consts.tile([P, K_TILES, HB], fp8)
            xnT_B = consts.tile([P, K_TILES, HB], fp8)
            xnT_NA = consts.tile([P, K_TILES, HB], fp8)
            xnT_NB = consts.tile([P, K_TILES, HB], fp8)
            maxacc = consts.tile([P, M_TILES, NCHUNK], f32)
            outt = consts.tile([P, M_TILES], f32)
            bias_log = consts.tile([P, 1], f32)
            nc.vector.memset(bias_log, 2.0 + _EPS_DIST)

            xnT_localA = dram.tile([D, HB], fp8)
            xnT_localB = dram.tile([D, HB], fp8)
            xnT_allA = dram.tile([ncores * D, HB], fp8, addr_space="Shared")
            xnT_allB = dram.tile([ncores * D, HB], fp8, addr_space="Shared")
            # ---- pre-pass: normalize*16, fp8-quantize, transpose, AG ----
            with (
                tc.tile_pool(name="prep", bufs=4) as prep,
                tc.tile_pool(name="ppsum", bufs=2, space="PSUM") as ppsum,
            ):
                # all 16 input loads are issued up front into distinct
                # buffers, fanned over three engine DMA queues: they
                # stream at full rate before AG_A's link traffic starts,
                # and the AG_A start is gated on the slowest core's
                # prep-A, so load latency directly delays the gathered
                # phase. mt 0-7: own shard (staged + AllGathered);
                # mt 8-15: the replicated neighbor shard (resident only).
                ld_engs = [nc.sync, nc.scalar, nc.gpsimd]
                xts = []
                for mt in range(2 * M_TILES):
                    m = mt % M_TILES
                    src_x = xs2 if mt >= M_TILES else xs
                    xt = prep.tile([P, D], f32, tag="xt", bufs=2 * M_TILES)
                    ld_engs[mt % 3].dma_start(xt, src_x[m * P : (m + 1) * P, :])
                    xts.append(xt)
                for mt in range(2 * M_TILES):
                    nbr = mt >= M_TILES
                    m = mt % M_TILES
                    xt = xts[mt]
                    sq = prep.tile([P, D], bf16, tag="sq")
                    ss = small.tile([P, 1], f32, tag="ss")
                    nc.scalar.activation(
                        sq,
                        xt,
                        mybir.ActivationFunctionType.Square,
                        accum_out=ss,
                    )
                    # norm/16 = sqrt(ss/256); then 16/norm via reciprocal
                    nrm = small.tile([P, 1], f32, tag="nrm")
                    nc.scalar.activation(
                        nrm,
                        ss,
                        mybir.ActivationFunctionType.Sqrt,
                        scale=1.0 / (_SCALE * _SCALE),
                    )
                    nrm2 = small.tile([P, 1], f32, tag="nrm2")
                    nc.vector.tensor_scalar_max(nrm2, nrm, _EPS_NORM)
                    rinv = small.tile([P, 1], f32, tag="rinv")
                    nc.vector.reciprocal(rinv, nrm2)
                    xnb = prep.tile([P, D], bf16, tag="xnb")
                    nc.vector.tensor_scalar_mul(xnb, xt, rinv)
                    pt = ppsum.tile([P, K_TILES, P], bf16, tag="pt")
                    for k in range(K_TILES):
                        nc.tensor.transpose(
                            pt[:, k, :], xnb[:, k * P : (k + 1) * P], identity
                        )
                    # one fp8-converting copy per m-tile, alternating DVE /
                    # scalar so neither engine serializes the prep pipeline
                    if nbr:
                        xnT_h = xnT_NA if m < MH else xnT_NB
                    else:
                        xnT_h = xnT_A if m < MH else xnT_B
                    m4 = m % MH
                    if m % 2 == 0:
                        nc.vector.tensor_copy(
                            xnT_h[:, :, m4 * P : (m4 + 1) * P], pt
                        )
                    else:
                        nc.scalar.activation(
                            xnT_h[:, :, m4 * P : (m4 + 1) * P],
                            pt,
                            mybir.ActivationFunctionType.Copy,
                        )
                    if nbr:
                        continue
                    if m == MH - 1:
                        for k in range(K_TILES):
                            (nc.sync if k % 2 == 0 else nc.scalar).dma_start(
                                xnT_localA[k * P : (k + 1) * P, :],
                                xnT_A[:, k, :],
                            )
                        nc.gpsimd.collective_compute(
                            "AllGather",
                            mybir.AluOpType.bypass,
                            replica_groups=[list(range(ncores))],
                            ins=[xnT_localA.opt()],
                            outs=[xnT_allA.opt()],
                        )
                    elif m == M_TILES - 1:
                        for k in range(K_TILES):
                            nc.sync.dma_start(
                                xnT_localB[k * P : (k + 1) * P, :],
                                xnT_B[:, k, :],
                            )
                        # AG_B is issued later, in the main pass, gated on
                        # the first gathered chunks' rhs tiles: launched
                        # back-to-back with AG_A it saturates the links
                        # exactly when the A-phase rhs reads start, and
                        # they crawl at ~30GB/s, stalling the first
                        # gathered matmuls by ~10us.

            # ---- main pass: 16 x 512-col Gram chunks, running row-max ----
            with (
                tc.tile_pool(name="rhsp", bufs=14) as rhsp,
                tc.tile_pool(name="mpsum", bufs=8, space="PSUM") as mpsum,
            ):
                order = [("own", 0), ("own", 1), ("nbr", 0), ("nbr", 1)]
                order += [(j, 0) for j in range(2, ncores)]
                order += [(j, 1) for j in range(2, ncores)]

                # -- DMA pass: issue every gathered rhs load up front.
                # Gathered columns are read with a core-relative rotation
                # (peer = (me+blk) & 7, blk=1..7) via a runtime-offset
                # DMA, so the core's own columns - already handled by the
                # masked resident pass - are never recomputed. Quarter-K
                # tiles per chunk, split over the gpsimd/scalar queues
                # (the sync queue's DMA engine contends with collective
                # traffic). All A-half loads are issued first so they
                # transfer in the quiet window right after AG_A lands;
                # AG_B is issued after them, gated on the first two
                # chunks' tiles, so its link traffic can't starve the
                # prefetch stream that feeds the first matmuls.
                rts_all = {}
                NQ = K_TILES // 2  # 4 quarter-K tiles per chunk (128KB
                # each) so the first matmul starts a quarter-transfer
                # after the AllGather lands instead of a half
                for idx, (blk, half) in enumerate(order):
                    if blk in ("own", "nbr"):
                        continue
                    src = xnT_allA if half == 0 else xnT_allB
                    rts = []
                    for q in range(NQ):
                        rt = rhsp.tile([P, 2, CHUNK], fp8, tag=f"rhs{q}")
                        dma_eng = nc.gpsimd if q % 2 == 0 else nc.scalar
                        pid = dma_eng.partition_id()
                        off = ((pid + blk) & 7) * D + q * 2 * P
                        dma_eng.dma_start(
                            rt,
                            src[bass_ds(off, 2 * P), :].rearrange(
                                "(k p) c -> p k c", k=2, p=P
                            ),
                        )
                        rts.append(rt)
                    rts_all[idx] = rts
                    if idx == 4 + (ncores - 2) - 1:
                        # all A-half loads in flight; rewrite row 0 of
                        # the B staging buffer with its own value, reading
                        # the second chunk's last tiles (per queue) first,
                        # so the AG_B trigger waits for the first two
                        # chunks' transfers
                        # gate copies run on gpsimd, NOT the vector
                        # engine: on the DVE queue they'd sit ahead of
                        # the chunk drains in FIFO order and block PSUM
                        # bank recycling (and so the resident-chunk
                        # matmuls) until AG_A lands
                        gate = small.tile([1, CHUNK], fp8, tag="gate")
                        nc.gpsimd.tensor_copy(gate, rts_all[4][0][0:1, 0, :])
                        nc.gpsimd.tensor_copy(gate, rts_all[4][1][0:1, 0, :])
                        nc.gpsimd.tensor_copy(gate, xnT_B[0:1, 0, :])
                        nc.sync.dma_start(xnT_localB[0:1, :], gate)
                        nc.gpsimd.collective_compute(
                            "AllGather",
                            mybir.AluOpType.bypass,
                            replica_groups=[list(range(ncores))],
                            ins=[xnT_localB.opt()],
                            outs=[xnT_allB.opt()],
                        )

                # -- compute pass --
                for idx, (blk, half) in enumerate(order):
                    own = blk == "own"
                    if own:
                        rts = [xnT_A if half == 0 else xnT_B] * 4
                        rk0 = [0, 2, 4, 6]
                    elif blk == "nbr":
                        rts = [xnT_NA if half == 0 else xnT_NB] * 4
                        rk0 = [0, 2, 4, 6]
                    else:
                        rts = rts_all[idx]
                        rk0 = [0, 0, 0, 0]
                    for m in range(M_TILES):
                        lhsT = xnT_A if m < MH else xnT_B
                        m4 = m % MH
                        ps = mpsum.tile([P, CHUNK], f32, tag="ps")
                        for kk in range(K_TILES // 2):
                            kb = rk0[kk]
                            nc.tensor.matmul(
                                ps,
                                lhsT[:, 2 * kk : 2 * kk + 2, m4 * P : (m4 + 1) * P],
                                rts[kk][:, kb : kb + 2, :],
                                start=(kk == 0),
                                stop=(kk == K_TILES // 2 - 1),
                                perf_mode=DR,
                            )
                        if own and (m // MH) == half:
                            off = m4 * P
                            nc.vector.tensor_add(
                                ps[:, off : off + P], ps[:, off : off + P], diagneg
                            )
                        # direct DVE f32 drain: 683ns/bank < 4-matmul bank
                        # refill (852ns+), so a single engine sustains the
                        # recycle rate; the old scalar-copy+bf16 alternate
                        # path had 1.4us serial latency and gated the PE.
                        nc.vector.reduce_max(
                            maxacc[:, m, idx : idx + 1],
                            ps,
                            axis=mybir.AxisListType.X,
                        )

                # ---- final: per-row max over the 16 chunk maxes (no
                # self-similarity is ever in the table: own columns come
                # only from the diag-masked resident pass), clamp,
                # 0.5*log(d^2). Emitted inside the main pool scope so the
                # pool-close drains land after the output DMA instead of
                # stalling the last chunk's matmuls.
                mxs = small.tile([P, M_TILES], f32, tag="mxs")
                nc.vector.reduce_max(mxs, maxacc, axis=mybir.AxisListType.X)
                mxc = small.tile([P, M_TILES], f32, tag="mxc")
                nc.vector.tensor_scalar_min(mxc, mxs, _SCALE * _SCALE)
                # ln(2 + eps - 2*sim) = 2*ln(dist); host multiplies 0.5
                nc.scalar.activation(
                    outt,
                    mxc,
                    mybir.ActivationFunctionType.Ln,
                    bias=bias_log,
                    scale=-2.0 / (_SCALE * _SCALE),
                )
                nc.sync.dma_start(out[:, :], outt)

    nc.compile()
    return nc


def _run(thought_vectors, trace=False, tmpdir=None):
    from concourse.bass_utils import run_bass_kernel_spmd

    ncores, NB, D = 8, 1024, 1024
    x = np.ascontiguousarray(
        np.asarray(thought_vectors, dtype=np.float32).reshape(-1, D)
    )
    N = x.shape[0]
    assert N == ncores * NB

    nc = _build_program(ncores, NB, D)

    in_maps = [
        {
            "xs": x[c * NB : (c + 1) * NB],
            "xs2": np.ascontiguousarray(
                x[((c + 1) % ncores) * NB : ((c + 1) % ncores + 1) * NB]
            ),
        }
        for c in range(ncores)
    ]

    res = run_bass_kernel_spmd(
        nc,
        in_maps,
        core_ids=list(range(ncores)),
        trace=trace,
        tmpdir=tmpdir,
    )

    total = 0.0
    for c in range(ncores):
        total += float(np.asarray(res.results[c]["out"], dtype=np.float64).sum())
    loss = -0.5 * total / N
    return np.float32(loss), res


def kernel(thought_vectors):
    loss, _ = _run(thought_vectors)
    return np.asarray(loss, dtype=np.float32)

